# revision 10
# baseline (speedup 1.0000x reference)
"""Trainium2 Bass kernel for int4-grouped-quantized linear (GPTQ-style).

out[8192, 11008] = x[8192, 4096] @ dequant(qweight, qzeros, scales)

Sharding: column-parallel over out_features N across 8 NeuronCores.
Each core dequantizes its W shard [4096, 1376] on-chip, loads x already
transposed via X-bar DMA-transpose (keeping the PE free for matmuls), and
runs dense fp16 matmuls with fp32 PSUM accumulation.
"""

import sys

sys.path.insert(0, "/opt/trn_rl_repo")

from contextlib import ExitStack

import numpy as np

import concourse.bass as bass
from concourse import bacc
import concourse.tile as tile
from concourse import mybir
from concourse.bass_utils import run_bass_kernel_spmd

AOT = mybir.AluOpType
F16, I32, F32 = mybir.dt.float16, mybir.dt.int32, mybir.dt.float32

T, K, N = 8192, 4096, 11008
NCORES = 8
NS = N // NCORES  # 1376 out cols per core
CS = NS // 8  # 172 packed int32 cols per core
G = 32  # quant groups (group size 128 == one k-block)
KB = K // 128  # 32 k-blocks
TC = 256  # t rows per x-transpose chunk
NCH = T // TC  # 32 chunks
TSUB = TC // 128  # 2 output row-blocks per chunk
SEGS = [(0, 512), (512, 512), (1024, 352)]  # N segments (PSUM bank sized)


def _body(ctx, tc, xd, qwd, qzd, scd, outd, zscr):
    nc = tc.nc
    cpool = ctx.enter_context(tc.tile_pool(name="const", bufs=1))
    qpool = ctx.enter_context(tc.tile_pool(name="qwp", bufs=4))
    stpool = ctx.enter_context(tc.tile_pool(name="stage", bufs=2))
    wpool = ctx.enter_context(tc.tile_pool(name="w", bufs=KB))
    bcpool = ctx.enter_context(tc.tile_pool(name="bc", bufs=3))
    xtpool = ctx.enter_context(tc.tile_pool(name="xt", bufs=3))
    pspool = ctx.enter_context(tc.tile_pool(name="ps", bufs=2, space="PSUM"))
    opool = ctx.enter_context(tc.tile_pool(name="o", bufs=3))

    # Device W columns use nibble-plane-major order: device col j*CS + c holds
    # logical out col c*8 + j. The host permutes `scales` to match and
    # un-permutes the output columns, so unpack writes stay contiguous.

    # ---- unpack zero-points: qz [G, CS] i32 -> z [G, NS] f16, park in DRAM ----
    qz_t = cpool.tile([G, CS], I32)
    nc.gpsimd.dma_start(qz_t[:], qzd)
    z_stage = cpool.tile([G, NS], I32)
    for j in range(8):
        nc.vector.tensor_scalar(
            z_stage[:, j * CS : (j + 1) * CS], qz_t[:], 4 * j, 0xF,
            AOT.logical_shift_right, AOT.bitwise_and,
        )
    z_t = cpool.tile([G, NS], F16)
    nc.vector.tensor_copy(z_t[:], z_stage[:])
    nc.gpsimd.dma_start(zscr, z_t[:])

    # ---- dequantize W = (w4 - z) * s, one k-block (= one quant group) at a time ----
    w_tiles = []
    for b in range(KB):
        qw_t = qpool.tile([128, CS], I32)
        nc.gpsimd.dma_start(qw_t[:], qwd[b * 128 : (b + 1) * 128, :])
        w_stage = stpool.tile([128, NS], I32)
        for j in range(8):
            nc.vector.tensor_scalar(
                w_stage[:, j * CS : (j + 1) * CS], qw_t[:], 4 * j, 0xF,
                AOT.logical_shift_right, AOT.bitwise_and,
            )
        w_t = wpool.tile([128, NS], F16)
        nc.scalar.copy(w_t[:], w_stage[:])
        # replicate this group's zero/scale row across 128 partitions via DMA
        z_bc = bcpool.tile([128, NS], F16, tag="zbc")
        nc.scalar.dma_start(z_bc[:], zscr[b : b + 1, :].partition_broadcast(128))
        s_bc = bcpool.tile([128, NS], F16, tag="sbc")
        nc.scalar.dma_start(s_bc[:], scd[b : b + 1, :].partition_broadcast(128))
        nc.vector.tensor_tensor(w_t[:], w_t[:], z_bc[:], AOT.subtract)
        nc.vector.tensor_tensor(w_t[:], w_t[:], s_bc[:], AOT.mult)
        w_tiles.append(w_t)

    # ---- x arrives transposed via X-bar DMA; PE does only matmuls ----
    # One transpose instruction per chunk: [TC, K] DRAM -> [128, KB, TC] SBUF,
    # xt[p, b, t] = x[r0 + t, b*128 + p].
    for c in range(NCH):
        r0 = c * TC
        xt = xtpool.tile([128, KB, TC], F16, tag="xt")
        nc.sync.dma_start_transpose(xt[:], xd[r0 : r0 + TC, :])
        for tsub in range(TSUB):
            ps = pspool.tile([128, NS], F32)
            for b in range(KB):
                st = xt[:, b, tsub * 128 : (tsub + 1) * 128]
                for off, sz in SEGS:
                    nc.tensor.matmul(
                        ps[:, off : off + sz],
                        st,
                        w_tiles[b][:, off : off + sz],
                        start=(b == 0),
                        stop=(b == KB - 1),
                    )
            ob = opool.tile([128, NS], F16)
            for off, sz in SEGS:
                nc.any.tensor_copy(ob[:, off : off + sz], ps[:, off : off + sz])
            ro = r0 + tsub * 128
            nc.gpsimd.dma_start(outd[ro : ro + 128, :], ob[:])


def build_kernel():
    nc = bacc.Bacc("TRN2", target_bir_lowering=False, debug=False)
    xd = nc.dram_tensor("x", [T, K], F16, kind="ExternalInput").ap()
    qwd = nc.dram_tensor("qw", [K, CS], I32, kind="ExternalInput").ap()
    qzd = nc.dram_tensor("qz", [G, CS], I32, kind="ExternalInput").ap()
    scd = nc.dram_tensor("sc", [G, NS], F16, kind="ExternalInput").ap()
    outd = nc.dram_tensor("out", [T, NS], F16, kind="ExternalOutput").ap()
    zscr = nc.dram_tensor("z_scratch", [G, NS], F16, kind="Internal").ap()
    with tile.TileContext(nc) as tc, ExitStack() as ctx:
        _body(ctx, tc, xd, qwd, qzd, scd, outd, zscr)
    nc.compile()
    return nc


_NC = None


def _get_nc():
    global _NC
    if _NC is None:
        _NC = build_kernel()
    return _NC


# device col n' = j*CS + c  <->  logical col n = c*8 + j (nibble-plane-major)
_N = np.arange(NS)
_PERM = (_N % CS) * 8 + (_N // CS)  # logical col for each device col
_INV = (_N % 8) * CS + (_N // 8)  # device col for each logical col


def make_in_maps(x, qweight, qzeros, scales):
    x = np.asarray(x, dtype=np.float16)
    qweight = np.asarray(qweight, dtype=np.int32)
    qzeros = np.asarray(qzeros, dtype=np.int32)
    scales = np.asarray(scales, dtype=np.float16)
    in_maps = []
    for c in range(NCORES):
        in_maps.append(
            {
                "x": x,
                "qw": np.ascontiguousarray(qweight[:, c * CS : (c + 1) * CS]),
                "qz": np.ascontiguousarray(qzeros[:, c * CS : (c + 1) * CS]),
                "sc": np.ascontiguousarray(
                    scales[:, c * NS : (c + 1) * NS][:, _PERM]
                ),
            }
        )
    return in_maps


def run(in_maps, **kwargs):
    return run_bass_kernel_spmd(
        _get_nc(), in_maps, core_ids=list(range(NCORES)), **kwargs
    )


def kernel(x, qweight, qzeros, scales):
    res = run(make_in_maps(x, qweight, qzeros, scales))
    outs = [res.results[c]["out"][:, _INV] for c in range(NCORES)]
    return np.concatenate(outs, axis=1)


# revision 14
# speedup vs baseline: 1.0021x; 1.0021x over previous
"""Trainium2 Bass kernel for int4-grouped-quantized linear (GPTQ-style).

out[8192, 11008] = x[8192, 4096] @ dequant(qweight, qzeros, scales)

Sharding: column-parallel over out_features N across 8 NeuronCores.
Each core dequantizes its W shard [4096, 1376] on-chip, loads x already
transposed via X-bar DMA-transpose (keeping the PE free for matmuls), and
runs dense fp16 matmuls with fp32 PSUM accumulation.
"""

import sys

sys.path.insert(0, "/opt/trn_rl_repo")

from contextlib import ExitStack

import numpy as np

import concourse.bass as bass
from concourse import bacc
import concourse.tile as tile
from concourse import mybir
from concourse.bass_utils import run_bass_kernel_spmd

AOT = mybir.AluOpType
F16, I32, F32 = mybir.dt.float16, mybir.dt.int32, mybir.dt.float32

T, K, N = 8192, 4096, 11008
NCORES = 8
NS = N // NCORES  # 1376 out cols per core
CS = NS // 8  # 172 packed int32 cols per core
G = 32  # quant groups (group size 128 == one k-block)
KB = K // 128  # 32 k-blocks
TC = 256  # t rows per x-transpose chunk
NCH = T // TC  # 32 chunks
TSUB = TC // 128  # 2 output row-blocks per chunk
SEGS = [(0, 512), (512, 512), (1024, 352)]  # N segments (PSUM bank sized)


def _body(ctx, tc, xd, qwd, qzd, scd, outd, zscr):
    nc = tc.nc
    cpool = ctx.enter_context(tc.tile_pool(name="const", bufs=1))
    qpool = ctx.enter_context(tc.tile_pool(name="qwp", bufs=4))
    stpool = ctx.enter_context(tc.tile_pool(name="stage", bufs=2))
    wpool = ctx.enter_context(tc.tile_pool(name="w", bufs=KB))
    bcpool = ctx.enter_context(tc.tile_pool(name="bc", bufs=3))
    xtpool = ctx.enter_context(tc.tile_pool(name="xt", bufs=3))
    pspool = ctx.enter_context(tc.tile_pool(name="ps", bufs=2, space="PSUM"))
    opool = ctx.enter_context(tc.tile_pool(name="o", bufs=3))

    # Device W columns use nibble-plane-major order: device col j*CS + c holds
    # logical out col c*8 + j. The host permutes `scales` to match and
    # un-permutes the output columns, so unpack writes stay contiguous.

    # ---- unpack zero-points: qz [G, CS] i32 -> z [G, NS] f16, park in DRAM ----
    qz_t = cpool.tile([G, CS], I32)
    nc.gpsimd.dma_start(qz_t[:], qzd)
    z_stage = cpool.tile([G, NS], I32)
    for j in range(8):
        nc.vector.tensor_scalar(
            z_stage[:, j * CS : (j + 1) * CS], qz_t[:], 4 * j, 0xF,
            AOT.logical_shift_right, AOT.bitwise_and,
        )
    z_t = cpool.tile([G, NS], F16)
    nc.vector.tensor_copy(z_t[:], z_stage[:])
    nc.gpsimd.dma_start(zscr, z_t[:])

    # ---- dequantize W = (w4 - z) * s, one k-block (= one quant group) at a time ----
    w_tiles = []
    for b in range(KB):
        qw_t = qpool.tile([128, CS], I32)
        nc.gpsimd.dma_start(qw_t[:], qwd[b * 128 : (b + 1) * 128, :])
        w_stage = stpool.tile([128, NS], I32)
        for j in range(8):
            nc.vector.tensor_scalar(
                w_stage[:, j * CS : (j + 1) * CS], qw_t[:], 4 * j, 0xF,
                AOT.logical_shift_right, AOT.bitwise_and,
            )
        w_t = wpool.tile([128, NS], F16)
        nc.scalar.copy(w_t[:], w_stage[:])
        # replicate this group's zero/scale row across 128 partitions via DMA
        z_bc = bcpool.tile([128, NS], F16, tag="zbc")
        nc.gpsimd.dma_start(z_bc[:], zscr[b : b + 1, :].partition_broadcast(128))
        s_bc = bcpool.tile([128, NS], F16, tag="sbc")
        nc.gpsimd.dma_start(s_bc[:], scd[b : b + 1, :].partition_broadcast(128))
        nc.vector.tensor_tensor(w_t[:], w_t[:], z_bc[:], AOT.subtract)
        nc.vector.tensor_tensor(w_t[:], w_t[:], s_bc[:], AOT.mult)
        w_tiles.append(w_t)

    # ---- x arrives transposed via X-bar DMA; PE does only matmuls ----
    # One transpose instruction per chunk: [TC, K] DRAM -> [128, KB, TC] SBUF,
    # xt[p, b, t] = x[r0 + t, b*128 + p].
    for c in range(NCH):
        r0 = c * TC
        xt = xtpool.tile([128, KB, TC], F16, tag="xt")
        nc.sync.dma_start_transpose(xt[:], xd[r0 : r0 + TC, :])
        if c == 0:
            # b-outer over both row-blocks: the PE makes progress on two
            # chains while the W dequant stream is still producing tiles.
            pss = [
                pspool.tile([128, NS], F32, name=f"ps0_{i}", tag="ps")
                for i in range(TSUB)
            ]
            for b in range(KB):
                for tsub in range(TSUB):
                    st = xt[:, b, tsub * 128 : (tsub + 1) * 128]
                    for off, sz in SEGS:
                        nc.tensor.matmul(
                            pss[tsub][:, off : off + sz],
                            st,
                            w_tiles[b][:, off : off + sz],
                            start=(b == 0),
                            stop=(b == KB - 1),
                        )
            for tsub in range(TSUB):
                ob = opool.tile([128, NS], F16)
                for off, sz in SEGS:
                    nc.any.tensor_copy(
                        ob[:, off : off + sz], pss[tsub][:, off : off + sz]
                    )
                ro = r0 + tsub * 128
                nc.gpsimd.dma_start(outd[ro : ro + 128, :], ob[:])
            continue
        for tsub in range(TSUB):
            ps = pspool.tile([128, NS], F32)
            for b in range(KB):
                st = xt[:, b, tsub * 128 : (tsub + 1) * 128]
                for off, sz in SEGS:
                    nc.tensor.matmul(
                        ps[:, off : off + sz],
                        st,
                        w_tiles[b][:, off : off + sz],
                        start=(b == 0),
                        stop=(b == KB - 1),
                    )
            ob = opool.tile([128, NS], F16)
            for off, sz in SEGS:
                nc.any.tensor_copy(ob[:, off : off + sz], ps[:, off : off + sz])
            ro = r0 + tsub * 128
            nc.gpsimd.dma_start(outd[ro : ro + 128, :], ob[:])


def build_kernel():
    nc = bacc.Bacc("TRN2", target_bir_lowering=False, debug=False)
    xd = nc.dram_tensor("x", [T, K], F16, kind="ExternalInput").ap()
    qwd = nc.dram_tensor("qw", [K, CS], I32, kind="ExternalInput").ap()
    qzd = nc.dram_tensor("qz", [G, CS], I32, kind="ExternalInput").ap()
    scd = nc.dram_tensor("sc", [G, NS], F16, kind="ExternalInput").ap()
    outd = nc.dram_tensor("out", [T, NS], F16, kind="ExternalOutput").ap()
    zscr = nc.dram_tensor("z_scratch", [G, NS], F16, kind="Internal").ap()
    with tile.TileContext(nc) as tc, ExitStack() as ctx:
        _body(ctx, tc, xd, qwd, qzd, scd, outd, zscr)
    nc.compile()
    return nc


_NC = None


def _get_nc():
    global _NC
    if _NC is None:
        _NC = build_kernel()
    return _NC


# device col n' = j*CS + c  <->  logical col n = c*8 + j (nibble-plane-major)
_N = np.arange(NS)
_PERM = (_N % CS) * 8 + (_N // CS)  # logical col for each device col
_INV = (_N % 8) * CS + (_N // 8)  # device col for each logical col


def make_in_maps(x, qweight, qzeros, scales):
    x = np.asarray(x, dtype=np.float16)
    qweight = np.asarray(qweight, dtype=np.int32)
    qzeros = np.asarray(qzeros, dtype=np.int32)
    scales = np.asarray(scales, dtype=np.float16)
    in_maps = []
    for c in range(NCORES):
        in_maps.append(
            {
                "x": x,
                "qw": np.ascontiguousarray(qweight[:, c * CS : (c + 1) * CS]),
                "qz": np.ascontiguousarray(qzeros[:, c * CS : (c + 1) * CS]),
                "sc": np.ascontiguousarray(
                    scales[:, c * NS : (c + 1) * NS][:, _PERM]
                ),
            }
        )
    return in_maps


def run(in_maps, **kwargs):
    return run_bass_kernel_spmd(
        _get_nc(), in_maps, core_ids=list(range(NCORES)), **kwargs
    )


def kernel(x, qweight, qzeros, scales):
    res = run(make_in_maps(x, qweight, qzeros, scales))
    outs = [res.results[c]["out"][:, _INV] for c in range(NCORES)]
    return np.concatenate(outs, axis=1)


# revision 15
# speedup vs baseline: 1.0302x; 1.0281x over previous
"""Trainium2 Bass kernel for int4-grouped-quantized linear (GPTQ-style).

out[8192, 11008] = x[8192, 4096] @ dequant(qweight, qzeros, scales)

Sharding: column-parallel over out_features N across 8 NeuronCores.

Per core: x arrives transposed via X-bar DMA-transpose (one 2MB instruction
per 256-row chunk), W is dequantized on-chip in three column "waves" so the
PE can start consuming partially-dequantized W while the unpack stream is
still running, and the matmuls accumulate fp16 x fp16 -> fp32 PSUM.

Device W columns are nibble-plane-major (device col j*CS + c holds logical
out col c*8 + j) so the int4 unpack writes contiguously; the host permutes
`scales`/zero-points to match and un-permutes output columns.

The zero/scale rows are pre-broadcast across partitions on the host (one
[128, 2*seg] fp16 block per (wave, k-block)) so the kernel loads them with
plain contiguous HWDGE DMAs instead of slow SWDGE partition-broadcasts.
"""

import sys

sys.path.insert(0, "/opt/trn_rl_repo")

from contextlib import ExitStack

import numpy as np

import concourse.bass as bass
from concourse import bacc
import concourse.tile as tile
from concourse import mybir
from concourse.bass_utils import run_bass_kernel_spmd

AOT = mybir.AluOpType
F16, I32, F32 = mybir.dt.float16, mybir.dt.int32, mybir.dt.float32

T, K, N = 8192, 4096, 11008
NCORES = 8
NS = N // NCORES  # 1376 out cols per core
CS = NS // 8  # 172 packed int32 cols per core
G = 32  # quant groups (group size 128 == one k-block)
KB = K // 128  # 32 k-blocks
TC = 256  # t rows per x-transpose chunk
NCH = T // TC  # 32 chunks
TSUB = TC // 128  # 2 output row-blocks per chunk
SEGS = [(0, 512), (512, 512), (1024, 352)]  # N segments (PSUM bank sized)
# wave w: shift planes PLANES[w], then apply (w4-z)*s on SEGS[w]
PLANES = [(0, 3), (3, 3), (6, 2)]  # (first plane, count)
NWCH = 4  # chunks consumed seg-wise during the dequant waves


def _body(ctx, tc, xd, qwd, zsd, outd):
    nc = tc.nc
    qpool = ctx.enter_context(tc.tile_pool(name="qwp", bufs=KB))
    stpool = ctx.enter_context(tc.tile_pool(name="stage", bufs=3))
    wpool = ctx.enter_context(tc.tile_pool(name="w", bufs=KB))
    zpool = ctx.enter_context(tc.tile_pool(name="zs", bufs=4))
    xtpool = ctx.enter_context(tc.tile_pool(name="xt", bufs=NWCH))
    pspool = ctx.enter_context(tc.tile_pool(name="ps", bufs=8, space="PSUM"))
    opool = ctx.enter_context(tc.tile_pool(name="o", bufs=8))

    # resident packed weights and fp16 W tiles
    qw_ts = []
    for b in range(KB):
        qw_t = qpool.tile([128, CS], I32, name=f"qw{b}", tag="qw")
        nc.gpsimd.dma_start(qw_t[:], qwd[b * 128 : (b + 1) * 128, :])
        qw_ts.append(qw_t)
    w_ts = [
        wpool.tile([128, NS], F16, name=f"w{b}", tag="w") for b in range(KB)
    ]

    # x-transpose chunks for the wave phase
    xts = {}
    for c in range(NWCH):
        r0 = c * TC
        xt = xtpool.tile([128, KB, TC], F16, name=f"xt{c}", tag="xt")
        nc.sync.dma_start_transpose(xt[:], xd[r0 : r0 + TC, :])
        xts[c] = xt

    def seg_chain(xt, c, tsub, si, w_slices):
        off, sz = SEGS[si]
        ps = pspool.tile(
            [128, sz], F32, name=f"ps_{c}_{tsub}_{si}", tag="ps"
        )
        for b in range(KB):
            st = xt[:, b, tsub * 128 : (tsub + 1) * 128]
            nc.tensor.matmul(
                ps[:], st, w_slices[b], start=(b == 0), stop=(b == KB - 1)
            )
        ob = opool.tile([128, sz], F16, name=f"ob_{c}_{tsub}_{si}", tag="ob")
        nc.any.tensor_copy(ob[:], ps[:])
        ro = c * TC + tsub * 128
        nc.gpsimd.dma_start(outd[ro : ro + 128, off : off + sz], ob[:])

    # ---- dequant waves: unpack planes, apply (w4 - z) * s per segment ----
    for si in range(3):
        off, sz = SEGS[si]
        p0, np_ = PLANES[si]
        for b in range(KB):
            stage = stpool.tile([128, np_ * CS], I32, name=f"st{si}_{b}", tag="st")
            for jj in range(np_):
                j = p0 + jj
                nc.vector.tensor_scalar(
                    stage[:, jj * CS : (jj + 1) * CS], qw_ts[b][:], 4 * j, 0xF,
                    AOT.logical_shift_right, AOT.bitwise_and,
                )
            nc.scalar.copy(w_ts[b][:, p0 * CS : (p0 + np_) * CS], stage[:])
            zst = zpool.tile([128, 2 * sz], F16, name=f"zs{si}_{b}", tag="zs")
            nc.sync.dma_start(
                zst[:], zsd[si][b * 128 : (b + 1) * 128, : 2 * sz]
            )
            wseg = w_ts[b][:, off : off + sz]
            nc.vector.tensor_tensor(wseg, wseg, zst[:, 0:sz], AOT.subtract)
            nc.vector.tensor_tensor(wseg, wseg, zst[:, sz : 2 * sz], AOT.mult)
        w_slices = [w_ts[b][:, off : off + sz] for b in range(KB)]
        for c in range(NWCH):
            for tsub in range(TSUB):
                seg_chain(xts[c], c, tsub, si, w_slices)

    # ---- steady chunks ----
    w_seg_slices = [
        [w_ts[b][:, off : off + sz] for b in range(KB)] for off, sz in SEGS
    ]
    for c in range(NWCH, NCH):
        r0 = c * TC
        xt = xtpool.tile([128, KB, TC], F16, name=f"xt{c}", tag="xt")
        nc.sync.dma_start_transpose(xt[:], xd[r0 : r0 + TC, :])
        for tsub in range(TSUB):
            pss = []
            for si, (off, sz) in enumerate(SEGS):
                ps = pspool.tile(
                    [128, sz], F32, name=f"ps_{c}_{tsub}_{si}", tag="ps"
                )
                pss.append(ps)
            for b in range(KB):
                st = xt[:, b, tsub * 128 : (tsub + 1) * 128]
                for si, (off, sz) in enumerate(SEGS):
                    nc.tensor.matmul(
                        pss[si][:],
                        st,
                        w_seg_slices[si][b],
                        start=(b == 0),
                        stop=(b == KB - 1),
                    )
            for si, (off, sz) in enumerate(SEGS):
                ob = opool.tile(
                    [128, sz], F16, name=f"ob_{c}_{tsub}_{si}", tag="ob"
                )
                nc.any.tensor_copy(ob[:], pss[si][:])
                ro = r0 + tsub * 128
                nc.gpsimd.dma_start(
                    outd[ro : ro + 128, off : off + sz], ob[:]
                )


def build_kernel():
    nc = bacc.Bacc("TRN2", target_bir_lowering=False, debug=False)
    xd = nc.dram_tensor("x", [T, K], F16, kind="ExternalInput").ap()
    qwd = nc.dram_tensor("qw", [K, CS], I32, kind="ExternalInput").ap()
    zsd = [
        nc.dram_tensor(
            f"zs{si}", [KB * 128, 2 * sz], F16, kind="ExternalInput"
        ).ap()
        for si, (off, sz) in enumerate(SEGS)
    ]
    outd = nc.dram_tensor("out", [T, NS], F16, kind="ExternalOutput").ap()
    with tile.TileContext(nc) as tc, ExitStack() as ctx:
        _body(ctx, tc, xd, qwd, zsd, outd)
    nc.compile()
    return nc


_NC = None


def _get_nc():
    global _NC
    if _NC is None:
        _NC = build_kernel()
    return _NC


# device col n' = j*CS + c  <->  logical col n = c*8 + j (nibble-plane-major)
_N = np.arange(NS)
_PERM = (_N % CS) * 8 + (_N // CS)  # logical col for each device col
_INV = (_N % 8) * CS + (_N // 8)  # device col for each logical col


def _unpack_u4(packed):
    shifts = np.arange(8, dtype=np.int32) * 4
    nib = (packed[:, :, None] >> shifts) & 0xF
    return nib.reshape(packed.shape[0], -1)


def make_in_maps(x, qweight, qzeros, scales):
    x = np.asarray(x, dtype=np.float16)
    qweight = np.asarray(qweight, dtype=np.int32)
    qzeros = np.asarray(qzeros, dtype=np.int32)
    scales = np.asarray(scales, dtype=np.float16)
    in_maps = []
    for c in range(NCORES):
        z_dev = _unpack_u4(qzeros[:, c * CS : (c + 1) * CS]).astype(np.float16)[
            :, _PERM
        ]
        s_dev = scales[:, c * NS : (c + 1) * NS][:, _PERM]
        m = {
            "x": x,
            "qw": np.ascontiguousarray(qweight[:, c * CS : (c + 1) * CS]),
        }
        for si, (off, sz) in enumerate(SEGS):
            zs = np.empty((KB, 128, 2, sz), dtype=np.float16)
            zs[:, :, 0, :] = z_dev[:, None, off : off + sz]
            zs[:, :, 1, :] = s_dev[:, None, off : off + sz]
            m[f"zs{si}"] = zs.reshape(KB * 128, 2 * sz)
        in_maps.append(m)
    return in_maps


def run(in_maps, **kwargs):
    return run_bass_kernel_spmd(
        _get_nc(), in_maps, core_ids=list(range(NCORES)), **kwargs
    )


def kernel(x, qweight, qzeros, scales):
    res = run(make_in_maps(x, qweight, qzeros, scales))
    outs = [res.results[c]["out"][:, _INV] for c in range(NCORES)]
    return np.concatenate(outs, axis=1)


# revision 18
# speedup vs baseline: 1.0371x; 1.0066x over previous
"""Trainium2 Bass kernel for int4-grouped-quantized linear (GPTQ-style).

out[8192, 11008] = x[8192, 4096] @ dequant(qweight, qzeros, scales)

Sharding: column-parallel over out_features N across 8 NeuronCores.

Per core: x arrives transposed via X-bar DMA-transpose (one 2MB instruction
per 256-row chunk), W is dequantized on-chip in three column "waves" so the
PE can start consuming partially-dequantized W while the unpack stream is
still running, and the matmuls accumulate fp16 x fp16 -> fp32 PSUM.

Device W columns are nibble-plane-major (device col j*CS + c holds logical
out col c*8 + j) so the int4 unpack writes contiguously; the host permutes
`scales`/zero-points to match and un-permutes output columns.

The zero/scale rows are pre-broadcast across partitions on the host (one
[128, 2*seg] fp16 block per (wave, k-block)) so the kernel loads them with
plain contiguous HWDGE DMAs instead of slow SWDGE partition-broadcasts.
"""

import sys

sys.path.insert(0, "/opt/trn_rl_repo")

from contextlib import ExitStack

import numpy as np

import concourse.bass as bass
from concourse import bacc
import concourse.tile as tile
from concourse import mybir
from concourse.bass_utils import run_bass_kernel_spmd

AOT = mybir.AluOpType
F16, I32, F32 = mybir.dt.float16, mybir.dt.int32, mybir.dt.float32

T, K, N = 8192, 4096, 11008
NCORES = 8
NS = N // NCORES  # 1376 out cols per core
CS = NS // 8  # 172 packed int32 cols per core
G = 32  # quant groups (group size 128 == one k-block)
KB = K // 128  # 32 k-blocks
TC = 256  # t rows per x-transpose chunk
NCH = T // TC  # 32 chunks
TSUB = TC // 128  # 2 output row-blocks per chunk
SEGS = [(0, 512), (512, 512), (1024, 352)]  # N segments (PSUM bank sized)
# wave w: shift planes PLANES[w], then apply (w4-z)*s on SEGS[w]
PLANES = [(0, 3), (3, 3), (6, 2)]  # (first plane, count)
NWCH = 4  # chunks consumed seg-wise during the dequant waves


def _body(ctx, tc, xd, qwd, zsd, outd):
    nc = tc.nc
    qpool = ctx.enter_context(tc.tile_pool(name="qwp", bufs=1))
    stpool = ctx.enter_context(tc.tile_pool(name="stage", bufs=3))
    wpool = ctx.enter_context(tc.tile_pool(name="w", bufs=KB))
    zpool = ctx.enter_context(tc.tile_pool(name="zs", bufs=4))
    xtpool = ctx.enter_context(tc.tile_pool(name="xt", bufs=NWCH))
    pspool = ctx.enter_context(tc.tile_pool(name="ps", bufs=8, space="PSUM"))
    opool = ctx.enter_context(tc.tile_pool(name="o", bufs=8))

    # resident packed weights (one batched DMA) and fp16 W tiles
    qw_all = qpool.tile([128, KB, CS], I32, name="qw_all", tag="qw")
    nc.gpsimd.dma_start(
        qw_all[:], qwd.rearrange("(b p) c -> p b c", p=128)
    )
    w_ts = [
        wpool.tile([128, NS], F16, name=f"w{b}", tag="w") for b in range(KB)
    ]

    # x-transpose chunks for the wave phase
    xts = {}
    for c in range(NWCH):
        r0 = c * TC
        xt = xtpool.tile([128, KB, TC], F16, name=f"xt{c}", tag="xt")
        nc.sync.dma_start_transpose(xt[:], xd[r0 : r0 + TC, :])
        xts[c] = xt

    # ---- dequant waves: unpack planes, apply (w4 - z) * s per segment.
    # Matmuls are b-major across all wave chunks so the PE tracks the
    # dequant stream in lockstep instead of stalling per chain.
    for si in range(3):
        off, sz = SEGS[si]
        p0, np_ = PLANES[si]
        pss = {}
        for c in range(NWCH):
            for tsub in range(TSUB):
                pss[(c, tsub)] = pspool.tile(
                    [128, sz], F32, name=f"ps_{c}_{tsub}_{si}", tag="ps"
                )
        for b in range(KB):
            stage = stpool.tile([128, np_ * CS], I32, name=f"st{si}_{b}", tag="st")
            for jj in range(np_):
                j = p0 + jj
                nc.vector.tensor_scalar(
                    stage[:, jj * CS : (jj + 1) * CS], qw_all[:, b, :], 4 * j,
                    0xF, AOT.logical_shift_right, AOT.bitwise_and,
                )
            nc.scalar.copy(w_ts[b][:, p0 * CS : (p0 + np_) * CS], stage[:])
            zst = zpool.tile([128, 2 * sz], F16, name=f"zs{si}_{b}", tag="zs")
            nc.scalar.dma_start(
                zst[:], zsd[si][b * 128 : (b + 1) * 128, : 2 * sz]
            )
            wseg = w_ts[b][:, off : off + sz]
            nc.vector.tensor_tensor(wseg, wseg, zst[:, 0:sz], AOT.subtract)
            nc.vector.tensor_tensor(wseg, wseg, zst[:, sz : 2 * sz], AOT.mult)
            for c in range(NWCH):
                for tsub in range(TSUB):
                    st = xts[c][:, b, tsub * 128 : (tsub + 1) * 128]
                    nc.tensor.matmul(
                        pss[(c, tsub)][:], st, wseg,
                        start=(b == 0), stop=(b == KB - 1),
                    )
        for c in range(NWCH):
            for tsub in range(TSUB):
                ob = opool.tile(
                    [128, sz], F16, name=f"ob_{c}_{tsub}_{si}", tag="ob"
                )
                nc.vector.tensor_copy(ob[:], pss[(c, tsub)][:])
                ro = c * TC + tsub * 128
                nc.gpsimd.dma_start(
                    outd[ro : ro + 128, off : off + sz], ob[:]
                )

    # ---- steady chunks ----
    w_seg_slices = [
        [w_ts[b][:, off : off + sz] for b in range(KB)] for off, sz in SEGS
    ]
    for c in range(NWCH, NCH):
        r0 = c * TC
        xt = xtpool.tile([128, KB, TC], F16, name=f"xt{c}", tag="xt")
        nc.sync.dma_start_transpose(xt[:], xd[r0 : r0 + TC, :])
        for tsub in range(TSUB):
            pss = []
            for si, (off, sz) in enumerate(SEGS):
                ps = pspool.tile(
                    [128, sz], F32, name=f"ps_{c}_{tsub}_{si}", tag="ps"
                )
                pss.append(ps)
            for b in range(KB):
                st = xt[:, b, tsub * 128 : (tsub + 1) * 128]
                for si, (off, sz) in enumerate(SEGS):
                    nc.tensor.matmul(
                        pss[si][:],
                        st,
                        w_seg_slices[si][b],
                        start=(b == 0),
                        stop=(b == KB - 1),
                    )
            for si, (off, sz) in enumerate(SEGS):
                ob = opool.tile(
                    [128, sz], F16, name=f"ob_{c}_{tsub}_{si}", tag="ob"
                )
                nc.any.tensor_copy(ob[:], pss[si][:])
                ro = r0 + tsub * 128
                nc.gpsimd.dma_start(
                    outd[ro : ro + 128, off : off + sz], ob[:]
                )


def build_kernel():
    nc = bacc.Bacc("TRN2", target_bir_lowering=False, debug=False)
    xd = nc.dram_tensor("x", [T, K], F16, kind="ExternalInput").ap()
    qwd = nc.dram_tensor("qw", [K, CS], I32, kind="ExternalInput").ap()
    zsd = [
        nc.dram_tensor(
            f"zs{si}", [KB * 128, 2 * sz], F16, kind="ExternalInput"
        ).ap()
        for si, (off, sz) in enumerate(SEGS)
    ]
    outd = nc.dram_tensor("out", [T, NS], F16, kind="ExternalOutput").ap()
    with tile.TileContext(nc) as tc, ExitStack() as ctx:
        _body(ctx, tc, xd, qwd, zsd, outd)
    nc.compile()
    return nc


_NC = None


def _get_nc():
    global _NC
    if _NC is None:
        _NC = build_kernel()
    return _NC


# device col n' = j*CS + c  <->  logical col n = c*8 + j (nibble-plane-major)
_N = np.arange(NS)
_PERM = (_N % CS) * 8 + (_N // CS)  # logical col for each device col
_INV = (_N % 8) * CS + (_N // 8)  # device col for each logical col


def _unpack_u4(packed):
    shifts = np.arange(8, dtype=np.int32) * 4
    nib = (packed[:, :, None] >> shifts) & 0xF
    return nib.reshape(packed.shape[0], -1)


def make_in_maps(x, qweight, qzeros, scales):
    x = np.asarray(x, dtype=np.float16)
    qweight = np.asarray(qweight, dtype=np.int32)
    qzeros = np.asarray(qzeros, dtype=np.int32)
    scales = np.asarray(scales, dtype=np.float16)
    in_maps = []
    for c in range(NCORES):
        z_dev = _unpack_u4(qzeros[:, c * CS : (c + 1) * CS]).astype(np.float16)[
            :, _PERM
        ]
        s_dev = scales[:, c * NS : (c + 1) * NS][:, _PERM]
        m = {
            "x": x,
            "qw": np.ascontiguousarray(qweight[:, c * CS : (c + 1) * CS]),
        }
        for si, (off, sz) in enumerate(SEGS):
            zs = np.empty((KB, 128, 2, sz), dtype=np.float16)
            zs[:, :, 0, :] = z_dev[:, None, off : off + sz]
            zs[:, :, 1, :] = s_dev[:, None, off : off + sz]
            m[f"zs{si}"] = zs.reshape(KB * 128, 2 * sz)
        in_maps.append(m)
    return in_maps


def run(in_maps, **kwargs):
    return run_bass_kernel_spmd(
        _get_nc(), in_maps, core_ids=list(range(NCORES)), **kwargs
    )


def kernel(x, qweight, qzeros, scales):
    res = run(make_in_maps(x, qweight, qzeros, scales))
    outs = [res.results[c]["out"][:, _INV] for c in range(NCORES)]
    return np.concatenate(outs, axis=1)
